# revision 17
# baseline (speedup 1.0000x reference)
"""Trainium2 Bass kernel for nn_Block_88476326297957.

CLIP-style attention-pooling transformer block:
  - 128 cls queries attend over 196*128 = 25088 key/value tokens
  - layernorm -> Q/K/V projections (768x768) -> softmax(QK^T/8) attention
    (with the predictor gate reducing to exactly 0.5*attn since softmax over
    a singleton axis is identically 1) -> residual -> LN -> MLP -> residual.

Sharding: the 25088 kv tokens are split 3136/core across 8 NeuronCores.
Each core layernorms its token shard, projects K/V (bf16 matmuls, fp32
accumulate), computes scoresT = K_h q_h^T per head ([keys,128] tiles),
exponentiates without max subtraction (scores are O(1), exp is safe in
fp32), and accumulates [V|1]^T @ expT into PSUM, yielding per-head
numerator [64,128] and denominator [1,128] partial sums. A 400KB
AllReduce combines the partials; every core then finishes the (tiny)
128-token MLP identically and core 0's output is returned.
"""

import math
import sys
import types

import numpy as np
import ml_dtypes

# ---------------------------------------------------------------------------
# Problem constants (hardcoded per the harness contract)
# ---------------------------------------------------------------------------
DIM = 768
HEADS = 12
HD = 64
L = 196
N = 128
NCORES = 8
TOKENS = L * N              # 25088 kv tokens
TPC = TOKENS // NCORES      # 3136 tokens per core
EPS = 1e-5
ICH = DIM // 128            # 6 contraction chunks of 128


def _ensure_ntff_hook():
    """Register the axon NTFF profiling hook if the image's antenv lacks it.

    Harmless when profiling is never requested; required for trace=True.
    """
    if "antenv.axon_hooks" in sys.modules:
        return
    mod = types.ModuleType("antenv.axon_hooks")
    _hook = [None]
    mod.set_axon_ntff_profile_hook = lambda h: _hook.__setitem__(0, h)
    mod.get_axon_ntff_profile_hook = lambda: _hook[0]
    sys.modules["antenv.axon_hooks"] = mod
    try:
        import antenv

        antenv.axon_hooks = mod
        from trn_agent_boot.trn_boot import _ntff_profile_via_ctypes

        mod.set_axon_ntff_profile_hook(
            _ntff_profile_via_ctypes("/opt/axon/libaxon_pjrt.so")
        )
    except Exception:
        pass


def _macro_tiles(tpc):
    """Token macro-tiles: multiples of 512 plus remainder, as (offset, size)."""
    tiles = []
    off = 0
    while off < tpc:
        sz = min(512, tpc - off)
        tiles.append((off, sz))
        off += sz
    return tiles


def build(tpc=TPC):
    """Build the Bass module (one program, run SPMD on 8 cores)."""
    import concourse.tile as tile
    from concourse import bacc, mybir
    from concourse.masks import make_identity

    f32 = mybir.dt.float32
    f32r = mybir.dt.float32r
    bf16 = mybir.dt.bfloat16

    nc = bacc.Bacc("TRN2", target_bir_lowering=False, debug=False,
                   num_devices=NCORES)

    xs = nc.declare_dram_parameter("xs", [tpc, DIM], bf16, isOutput=False)
    cls_d = nc.declare_dram_parameter("cls", [N, DIM], f32, isOutput=False)
    # [w(q,k,v), ichunk, p, o] with element = W[o, ic*128+p] * g1[ic*128+p]
    wqkv_d = nc.declare_dram_parameter("wqkvT", [3, ICH, 128, DIM], bf16,
                                       isOutput=False)
    # [w(fc,proj), ichunk, p, o] fp32, g2 folded into fc
    mlp_d = nc.declare_dram_parameter("mlpT", [2, ICH, 128, DIM], f32r,
                                      isOutput=False)
    mlpb_d = nc.declare_dram_parameter("mlp_b", [2, DIM], f32r, isOutput=False)
    out_d = nc.declare_dram_parameter("out", [N, DIM], f32, isOutput=True)
    import os as _os
    _dbg = bool(_os.environ.get("KERNEL_DEBUG"))
    if _dbg:
        dbg_q0 = nc.declare_dram_parameter("dbg_q0", [N, DIM], f32, isOutput=True)
        dbg_ctx = nc.declare_dram_parameter("dbg_ctx", [HD + 1, HEADS * 128], bf16,
                                            isOutput=True)
        dbg_ctxf = nc.declare_dram_parameter("dbg_ctxf", [N, DIM], f32, isOutput=True)
        dbg_q1 = nc.declare_dram_parameter("dbg_q1", [N, DIM], f32, isOutput=True)
        dbg_m1 = nc.declare_dram_parameter("dbg_m1", [N, DIM], f32, isOutput=True)
        dbg_m2 = nc.declare_dram_parameter("dbg_m2", [N, DIM], f32, isOutput=True)

    mts = _macro_tiles(tpc)
    n_sub_total = sum((sz + 127) // 128 for _, sz in mts)

    with tile.TileContext(nc) as tc:
        with (
            tc.tile_pool(name="singles", bufs=1) as singles,
            tc.tile_pool(name="ctxps", bufs=3, space="PSUM") as ctxps,
            tc.tile_pool(name="dram", bufs=2, space="DRAM") as dram,
        ):
            # ---- resident weights & constants -------------------------------
            ident_bf = singles.tile([128, 128], bf16, tag="ident_bf")
            make_identity(nc, ident_bf)
            ident_f = singles.tile([128, 128], f32, tag="ident_f")
            make_identity(nc, ident_f)
            eps_sb = singles.tile([128, 1], f32, tag="eps")
            nc.vector.memset(eps_sb, EPS)
            ones1f = singles.tile([1, 128], f32, tag="ones1f")
            nc.vector.memset(ones1f, 1.0)
            ones1 = singles.tile([1, 128], f32r, tag="ones1")
            nc.vector.tensor_copy(out=ones1[:, :], in_=ones1f[:, :])

            wq = singles.tile([128, ICH, DIM], bf16, tag="wq")
            wk = singles.tile([128, ICH, DIM], bf16, tag="wk")
            wv = singles.tile([128, ICH, DIM], bf16, tag="wv")
            for w_t, wi in ((wq, 0), (wk, 1), (wv, 2)):
                for ic in range(ICH):
                    nc.gpsimd.dma_start(out=w_t[:, ic, :], in_=wqkv_d[wi, ic, :, :])
            wfc = singles.tile([128, ICH, DIM], f32r, tag="wfc")
            wpj = singles.tile([128, ICH, DIM], f32r, tag="wpj")
            fcb = singles.tile([1, DIM], f32r, tag="fcb")
            pjb = singles.tile([1, DIM], f32r, tag="pjb")

            def load_mlp_weights():
                # emitted mid-kernel so these 4.7MB don't compete with the
                # x/wqkv DMAs during the ramp
                for w_t, wi in ((wfc, 0), (wpj, 1)):
                    for ic in range(ICH):
                        nc.gpsimd.dma_start(out=w_t[:, ic, :],
                                            in_=mlp_d[wi, ic, :, :])
                nc.gpsimd.dma_start(out=fcb[:, :], in_=mlpb_d[0:1, :])
                nc.gpsimd.dma_start(out=pjb[:, :], in_=mlpb_d[1:2, :])

            # persistent across phase 2+3
            q0 = singles.tile([N, DIM], f32, tag="q0")
            qT = singles.tile([128, ICH, 128], bf16, tag="qT")
            ctx_sb = singles.tile([128, HEADS * 128], bf16, tag="ctx_sb")

            # helper: layernorm stats -> (r, -mu*r) tiles
            def ln_stats(pool, src_ap, p):
                stats = pool.tile([128, 3, 6], f32, tag="stats")
                for sg in range(3):
                    nc.vector.bn_stats(
                        out=stats[:p, sg, :],
                        in_=src_ap[:, sg * 256:(sg + 1) * 256],
                    )
                mv = pool.tile([128, 2], f32, tag="mv")
                nc.vector.bn_aggr(out=mv[:p, :], in_=stats[:p, :, :])
                sd = pool.tile([128, 1], f32, tag="sd")
                nc.scalar.activation(out=sd[:p], in_=mv[:p, 1:2],
                                     func=mybir.ActivationFunctionType.Sqrt,
                                     bias=eps_sb[:p], scale=1.0)
                r = pool.tile([128, 1], f32, tag="r")
                nc.vector.reciprocal(out=r[:p], in_=sd[:p])
                nmr = pool.tile([128, 1], f32, tag="nmr")
                nc.vector.tensor_scalar(out=nmr[:p], in0=mv[:p, 0:1],
                                        scalar1=r[:p], scalar2=-1.0,
                                        op0=mybir.AluOpType.mult,
                                        op1=mybir.AluOpType.mult)
                return r, nmr

            with (
                tc.tile_pool(name="stats", bufs=4) as statsp,
                tc.tile_pool(name="ps", bufs=3, space="PSUM") as ps,
                tc.tile_pool(name="psbf", bufs=2, space="PSUM") as psbf,
                tc.tile_pool(name="xt", bufs=3) as xtp,
                tc.tile_pool(name="xln", bufs=2) as xlnp,
                tc.tile_pool(name="xlnT", bufs=2) as xlntp,
                tc.tile_pool(name="kt", bufs=2) as ktp,
                tc.tile_pool(name="vt", bufs=2) as vtp,
                tc.tile_pool(name="expp", bufs=3) as expp,
            ):
                # ---- phase 1: q0 = LN(cls);  qT[o, t] ----------------------
                cls_sb = xtp.tile([N, DIM], f32, tag="cls")
                nc.sync.dma_start(out=cls_sb[:, :], in_=cls_d[:, :])
                r, nmr = ln_stats(statsp, cls_sb[:, :], N)
                nc.scalar.activation(out=q0[:, :], in_=cls_sb[:, :],
                                     func=mybir.ActivationFunctionType.Identity,
                                     bias=nmr[:N], scale=r[:N])
                q0_bf = xlnp.tile([N, DIM], bf16, tag="q0bf")
                nc.vector.tensor_copy(out=q0_bf[:, :], in_=q0[:, :])
                q0T = xlntp.tile([128, ICH, 128], bf16, tag="q0T")
                for ic in range(ICH):
                    tp = psbf.tile([128, 512], bf16, tag="bigbf")
                    nc.tensor.transpose(tp[:, 0:128],
                                        q0_bf[:, ic * 128:(ic + 1) * 128],
                                        ident_bf[:, :])
                    nc.vector.tensor_copy(out=q0T[:, ic, :], in_=tp[:, 0:128])
                for oc in range(ICH):
                    acc = ps.tile([128, 512], f32, tag="big")
                    for ic in range(ICH):
                        nc.tensor.matmul(acc[:, 0:128],
                                         lhsT=wq[:, ic, oc * 128:(oc + 1) * 128],
                                         rhs=q0T[:, ic, :],
                                         start=(ic == 0), stop=(ic == ICH - 1))
                    nc.vector.tensor_copy(out=qT[:, oc, :], in_=acc[:, 0:128])

                # ---- phase 2: streaming attention over kv shard ------------
                ctx_ps = [ctxps.tile([128, 512], f32, tag="ctx", name=f"ctx{g}")
                          for g in range(3)]
                sub_idx = 0
                for mt0, mtsz in mts:
                    nsub = (mtsz + 127) // 128
                    x_t = xtp.tile([128, 4, DIM], bf16, tag="x")
                    if mtsz == 512:
                        nc.sync.dma_start(
                            out=x_t[:, :, :],
                            in_=xs[mt0:mt0 + 512, :].rearrange(
                                "(s p) o -> p s o", p=128),
                        )
                    else:
                        for s in range(nsub):
                            p = min(128, mtsz - s * 128)
                            nc.sync.dma_start(
                                out=x_t[:p, s, :],
                                in_=xs[mt0 + s * 128: mt0 + s * 128 + p, :])
                    xln = xlnp.tile([128, 4, DIM], bf16, tag="xln")
                    for s in range(nsub):
                        p = min(128, mtsz - s * 128)
                        r, nmr = ln_stats(statsp, x_t[:p, s, :], p)
                        nc.scalar.activation(
                            out=xln[:p, s, :], in_=x_t[:p, s, :],
                            func=mybir.ActivationFunctionType.Identity,
                            bias=nmr[:p], scale=r[:p])
                    # transpose -> xlnT [i, t]; batch 4 subtiles per psum
                    # tile so each ic needs a single evacuation copy
                    xlnT = xlntp.tile([128, ICH, 512], bf16, tag="xlnT")
                    for ic in range(ICH):
                        tp = psbf.tile([128, 512], bf16, tag="bigbf")
                        for s in range(nsub):
                            p = min(128, mtsz - s * 128)
                            nc.tensor.transpose(
                                tp[:, s * 128:s * 128 + p],
                                xln[:p, s, ic * 128:(ic + 1) * 128],
                                ident_bf[:p, :p])
                        nc.vector.tensor_copy(
                            out=xlnT[:, ic, 0:mtsz],
                            in_=tp[:, 0:mtsz])
                    # K^T [o, t]
                    kT = ktp.tile([128, ICH, 512], bf16, tag="kT")
                    for oc in range(ICH):
                        acc = ps.tile([128, 512], f32, tag="big")
                        for ic in range(ICH):
                            nc.tensor.matmul(
                                acc[:, 0:mtsz],
                                lhsT=wk[:, ic, oc * 128:(oc + 1) * 128],
                                rhs=xlnT[:, ic, 0:mtsz],
                                start=(ic == 0), stop=(ic == ICH - 1))
                        nc.vector.tensor_copy(out=kT[:, oc, 0:mtsz],
                                              in_=acc[:, 0:mtsz])
                    # V [t, o] interleaved with ones column -> [t, h, 65]
                    v_sb = vtp.tile([128, 4, HEADS, HD + 1], bf16, tag="v")
                    nc.vector.memset(v_sb[:, :, :, HD:HD + 1], 1.0)
                    for s in range(nsub):
                        p = min(128, mtsz - s * 128)
                        for half in range(2):
                            acc = ps.tile([128, 512], f32, tag="big")
                            osl = slice(half * 384, (half + 1) * 384)
                            for ic in range(ICH):
                                nc.tensor.matmul(
                                    acc[:p, 0:384],
                                    lhsT=xlnT[:, ic, s * 128:s * 128 + p],
                                    rhs=wv[:, ic, osl],
                                    start=(ic == 0), stop=(ic == ICH - 1))
                            nc.vector.tensor_copy(
                                out=v_sb[:p, s, half * 6:(half + 1) * 6, 0:HD],
                                in_=acc[:p, 0:384].rearrange(
                                    "p (h d) -> p h d", h=6))
                    # scores^T, exp, PV accumulate
                    for s in range(nsub):
                        p = min(128, mtsz - s * 128)
                        ssl = slice(s * 128, s * 128 + p)
                        # e_bf layout [p, parity, oc, q]: head h = 2*oc + parity
                        e_bf = expp.tile([128, 2, ICH, 128], bf16, tag="e")
                        for half in range(2):       # partition base parity
                            h_lo = 64 * half
                            for g in range(2):      # oc triples
                                sc = ps.tile([128, 384], f32, tag="big")
                                for j in range(3):
                                    oc = 3 * g + j
                                    nc.tensor.matmul(
                                        sc[:p, j * 128:(j + 1) * 128],
                                        lhsT=kT[h_lo:h_lo + 64, oc, ssl],
                                        rhs=qT[h_lo:h_lo + 64, oc, :],
                                        tile_position=(h_lo, 0),
                                        start=True, stop=True)
                                nc.scalar.activation(
                                    out=e_bf[:p, half, 3 * g:3 * g + 3, :],
                                    in_=sc[:p, 0:384].rearrange(
                                        "p (h q) -> p h q", h=3),
                                    func=mybir.ActivationFunctionType.Exp,
                                    scale=0.125)
                        first = sub_idx == 0
                        last = sub_idx == n_sub_total - 1
                        for h in range(HEADS):
                            # start=True resets has_written for the WHOLE psum
                            # bank: issue it only on the first write to each
                            # bank or it wipes sibling heads' accumulation.
                            nc.tensor.matmul(
                                ctx_ps[h // 4][0:HD + 1,
                                               (h % 4) * 128:(h % 4 + 1) * 128],
                                lhsT=v_sb[:p, s, h, :],
                                rhs=e_bf[:p, h % 2, h // 2, :],
                                start=(first and h % 4 == 0), stop=last,
                                skip_group_check=True)
                        sub_idx += 1

                load_mlp_weights()
                # evacuate ctx partials (bf16 for a smaller collective)
                for g in range(3):
                    nc.vector.tensor_copy(
                        out=ctx_sb[0:HD + 1, g * 512:(g + 1) * 512],
                        in_=ctx_ps[g][0:HD + 1, :])

            # ---- AllReduce partials ------------------------------------------
            cc_in = dram.tile([HD + 1, HEADS * 128], bf16, tag="cc_in")
            cc_out = dram.tile([NCORES * (HD + 1), HEADS * 128], bf16,
                               tag="cc_out")
            nc.sync.dma_start(out=cc_in[:, :], in_=ctx_sb[0:HD + 1, :])
            if _dbg:
                nc.sync.dma_start(out=dbg_q0[:, :], in_=q0[:, :])
                nc.sync.dma_start(out=dbg_ctx[:, :], in_=ctx_sb[0:HD + 1, :])
            nc.gpsimd.collective_compute(
                "AllGather", mybir.AluOpType.bypass,
                replica_groups=[list(range(NCORES))],
                ins=[cc_in.opt()], outs=[cc_out.opt()])

            # ---- phase 3: combine + MLP (replicated on all cores) -----------
            with (
                tc.tile_pool(name="fin", bufs=1) as fin,
                tc.tile_pool(name="stats3", bufs=4) as stats3,
                tc.tile_pool(name="ps3", bufs=4, space="PSUM") as ps3,
            ):
                red8 = fin.tile([128, NCORES, HEADS * 128], bf16, tag="red8")
                nc.sync.dma_start(
                    out=red8[0:HD + 1, :, :],
                    in_=cc_out[:, :].rearrange("(r p) f -> p r f", p=HD + 1))
                s4 = fin.tile([128, 4, HEADS * 128], f32, tag="s4")
                nc.vector.tensor_add(out=s4[0:HD + 1, :, :],
                                     in0=red8[0:HD + 1, 0:4, :],
                                     in1=red8[0:HD + 1, 4:8, :])
                red = fin.tile([128, HEADS * 128], f32, tag="red")
                nc.vector.tensor_add(out=s4[0:HD + 1, 0:2, :],
                                     in0=s4[0:HD + 1, 0:2, :],
                                     in1=s4[0:HD + 1, 2:4, :])
                nc.vector.tensor_add(out=red[0:HD + 1, :],
                                     in0=s4[0:HD + 1, 0, :],
                                     in1=s4[0:HD + 1, 1, :])
                ctxq = fin.tile([128, HEADS, HD + 1], f32, tag="ctxq")
                for h in range(HEADS):
                    tp = ps3.tile([128, 512], f32, tag="big3")
                    nc.tensor.transpose(
                        tp[:, 0:HD + 1],
                        red[0:HD + 1, h * 128:(h + 1) * 128],
                        ident_f[0:HD + 1, 0:HD + 1])
                    nc.vector.tensor_copy(out=ctxq[:, h, :], in_=tp[:, 0:HD + 1])
                ctxf = fin.tile([N, DIM], f32, tag="ctxf")
                rcp = fin.tile([128, HEADS], f32, tag="rcp")
                for h in range(HEADS):
                    nc.vector.reciprocal(out=rcp[:, h:h + 1],
                                         in_=ctxq[:, h, HD:HD + 1])
                    nc.vector.tensor_scalar(
                        out=ctxf[:, h * HD:(h + 1) * HD],
                        in0=ctxq[:, h, 0:HD],
                        scalar1=rcp[:, h:h + 1], scalar2=0.5,
                        op0=mybir.AluOpType.mult, op1=mybir.AluOpType.mult)
                q1 = fin.tile([N, DIM], f32, tag="q1")
                nc.vector.tensor_add(out=q1[:, :], in0=q0[:, :], in1=ctxf[:, :])
                if _dbg:
                    nc.sync.dma_start(out=dbg_ctxf[:, :], in_=ctxf[:, :])
                    nc.sync.dma_start(out=dbg_q1[:, :], in_=q1[:, :])
                # LN(q1) -> h
                r3, nmr3 = ln_stats(stats3, q1[:, :], N)
                h_sb = fin.tile([N, DIM], f32, tag="h")
                nc.scalar.activation(out=h_sb[:, :], in_=q1[:, :],
                                     func=mybir.ActivationFunctionType.Identity,
                                     bias=nmr3[:N], scale=r3[:N])

                def transpose6_f32(src, pool, tag):
                    dst = pool.tile([128, ICH, 128], f32r, tag=tag, name=tag)
                    for ic in range(ICH):
                        tp = ps3.tile([128, 512], f32, tag="big3")
                        nc.tensor.transpose(tp[:, 0:128],
                                            src[:, ic * 128:(ic + 1) * 128],
                                            ident_f[:, :])
                        nc.vector.tensor_copy(out=dst[:, ic, :], in_=tp[:, 0:128])
                    return dst

                def mlp_layer(inpT, w_t, bias_row, pool, name):
                    """out[t, o] = inpT.T @ w + bias ; returns psum tiles."""
                    outs = []
                    for half in range(2):
                        acc = ps3.tile([128, 512], f32, tag="big3")
                        osl = slice(half * 384, (half + 1) * 384)
                        nc.tensor.matmul(
                            acc[:, 0:384],
                            lhsT=ones1[0:1, :],
                            rhs=bias_row[:, osl],
                            start=True, stop=False)
                        for ic in range(ICH):
                            nc.tensor.matmul(
                                acc[:, 0:384],
                                lhsT=inpT[:, ic, :],
                                rhs=w_t[:, ic, osl],
                                start=False, stop=(ic == ICH - 1))
                        outs.append(acc)
                    return outs

                hT = transpose6_f32(h_sb, fin, "hT")
                m1ps = mlp_layer(hT, wfc, fcb, fin, "fc")
                m1 = fin.tile([N, DIM], f32, tag="m1")
                sig = fin.tile([N, DIM], f32, tag="sig")
                for half in range(2):
                    osl = slice(half * 384, (half + 1) * 384)
                    nc.vector.tensor_copy(out=m1[:, osl], in_=m1ps[half][:, 0:384])
                    nc.scalar.activation(out=sig[:, osl], in_=m1ps[half][:, 0:384],
                                         func=mybir.ActivationFunctionType.Sigmoid,
                                         scale=1.702)
                m2 = fin.tile([N, DIM], f32, tag="m2")
                nc.vector.tensor_mul(out=m2[:, :], in0=m1[:, :], in1=sig[:, :])
                if _dbg:
                    nc.sync.dma_start(out=dbg_m1[:, :], in_=m1[:, :])
                    nc.sync.dma_start(out=dbg_m2[:, :], in_=m2[:, :])
                m2T = transpose6_f32(m2, fin, "m2T")
                m3ps = mlp_layer(m2T, wpj, pjb, fin, "proj")
                out_sb = fin.tile([N, DIM], f32, tag="out")
                for half in range(2):
                    osl = slice(half * 384, (half + 1) * 384)
                    nc.vector.tensor_add(out=out_sb[:, osl], in0=q1[:, osl],
                                         in1=m3ps[half][:, 0:384])
                nc.sync.dma_start(out=out_d[:, :], in_=out_sb[:, :])

    nc.compile()
    return nc


_BUILD_CACHE = {}


def _get_nc(tpc=TPC):
    if tpc not in _BUILD_CACHE:
        _BUILD_CACHE[tpc] = build(tpc)
    return _BUILD_CACHE[tpc]


def prep_inputs(x, cls, g1, b1, g2, b2, Wq, Wk, Wv, fc_w, fc_b, proj_w, proj_b,
                tpc=TPC):
    """Host-side sharding + weight prep. Returns per-core input maps."""
    x = np.asarray(x, np.float32)
    cls = np.asarray(cls, np.float32)
    g1 = np.asarray(g1, np.float32)
    b1 = np.asarray(b1, np.float32)
    g2 = np.asarray(g2, np.float32)
    b2 = np.asarray(b2, np.float32)
    assert np.allclose(b1, 0.0), "nonzero b1 not supported by this build"
    xs = x.reshape(L * N, DIM)
    cls2 = cls.reshape(N, DIM)
    if not np.allclose(g1, 1.0):
        # g1 folds into the QKV weights; the q0 residual path also needs it,
        # which this build does not implement.
        raise NotImplementedError("non-unit g1")

    def foldT(w, g):
        return np.ascontiguousarray((np.asarray(w, np.float32) * g[None, :]).T)

    wqkvT = np.stack([
        foldT(Wq, g1).astype(ml_dtypes.bfloat16),
        foldT(Wk, g1).astype(ml_dtypes.bfloat16),
        foldT(Wv, g1).astype(ml_dtypes.bfloat16),
    ]).reshape(3, ICH, 128, DIM)
    mlpT = np.stack([
        foldT(fc_w, g2),
        np.ascontiguousarray(np.asarray(proj_w, np.float32).T),
    ]).reshape(2, ICH, 128, DIM)
    fc_b_eff = np.asarray(fc_b, np.float32) + np.asarray(fc_w, np.float32) @ b2
    mlp_b = np.stack([fc_b_eff, np.asarray(proj_b, np.float32)])

    in_maps = []
    for c in range(NCORES):
        in_maps.append({
            "xs": np.ascontiguousarray(xs[c * tpc:(c + 1) * tpc]).astype(
                ml_dtypes.bfloat16),
            "cls": cls2,
            "wqkvT": wqkvT,
            "mlpT": mlpT,
            "mlp_b": mlp_b,
        })
    return in_maps


def run(inputs, tpc=TPC, trace=False):
    _ensure_ntff_hook()
    from concourse.bass_utils import run_bass_kernel_spmd

    nc = _get_nc(tpc)
    in_maps = prep_inputs(
        inputs["x"], inputs["cls"], inputs["g1"], inputs["b1"], inputs["g2"],
        inputs["b2"], inputs["Wq"], inputs["Wk"], inputs["Wv"], inputs["fc_w"],
        inputs["fc_b"], inputs["proj_w"], inputs["proj_b"], tpc=tpc)
    res = run_bass_kernel_spmd(nc, in_maps, core_ids=list(range(NCORES)),
                               trace=trace)
    out = np.asarray(res.results[0]["out"], np.float32).reshape(1, N, DIM)
    return out, res


def kernel(**inputs):
    out, _ = run(inputs, tpc=TPC, trace=False)
    return out
